# revision 6
# baseline (speedup 1.0000x reference)
"""Patch-embedding kernel for Trainium2, data-parallel over batch on 8 NeuronCores.

Reference computation (per image):
  patches = im2col(image, 16x16)            # [196, 768]
  out = gelu(patches @ W + b, exact)        # [196, 768]

Sharding: batch 64 -> 8 images per core; host concatenates per-core outputs.

Layout strategy: im2col is a pure permutation for stride-16 non-overlapping
patches, so the HOST performs im2col + transpose + bf16 cast and uploads
X^T in k-major chunk layout xt[p, kc, m] = X[m, 128*kc + p]. Every device
DMA is then a wide contiguous read (>=1.5 KB per partition line) -- this
removes the 192-byte-fragment im2col gather and the on-device xbar
transposes that dominated the previous version.

Matmul orientation: transposed output. For each 128-wide n-chunk,
  psum[n, m] = sum_kc W[k, n].T @ X^T[k, m]
with W chunks as the stationary operand (natural layout, uploaded
pre-chunked) and X^T as the bf16 moving operand. Benefits:
  - no bias matmuls: bias is per-PARTITION in this orientation, applied for
    free by ScalarE as gelu(psum + bias[p]) during the PSUM->SBUF pass
  - m-tiles of 392 (=1568/4) tile M exactly; lhsT is always full 128x128
  - output stored bf16 (halves store traffic); host transposes + upcasts.

Per-core loop: for each m-tile (392 rows), for each of 6 n-chunks,
accumulate 6 k-chunk matmuls into one PSUM bank, then ScalarE applies
exact GELU (+bias) writing bf16, and the result is DMA'd out. 24 PSUM
groups rotate through all 8 banks; X^T arrives in 12 half-chunks so the
first m-tile's matmuls start after ~1.8 MB of loads instead of 3.6 MB.
"""

import numpy as np
import ml_dtypes

import concourse.bass as bass
import concourse.tile as tile
import concourse.mybir as mybir
from concourse import bacc
from concourse.bass_utils import run_bass_kernel_spmd

P = 16
D = 768
B, H, W, C = 64, 224, 224, 3
NH = NW = 14
NPATCH = NH * NW            # 196
K = P * P * C               # 768
NCORES = 8
BPC = B // NCORES           # 8 images per core
M = BPC * NPATCH            # 1568 output rows per core
KC = K // 128               # 6 k-chunks
NC6 = D // 128              # 6 n-chunks
MT = 392                    # m-tile (1568 = 4*392)
NMT = M // MT               # 4 m-tiles
MH = M // 2                 # 784, half-M granularity for the X^T loads

_BF16 = mybir.dt.bfloat16
_F32 = mybir.dt.float32


def _build():
    nc = bacc.Bacc("TRN2", target_bir_lowering=False, debug=False,
                   num_devices=NCORES)
    # Host-prepared layouts (see _run): all reads/writes contiguous.
    xt = nc.dram_tensor("xt", [128, KC, M], _BF16, kind="ExternalInput").ap()
    w = nc.dram_tensor("w", [128, KC, D], _BF16, kind="ExternalInput").ap()
    bias = nc.dram_tensor("bias", [128, NC6], _F32, kind="ExternalInput").ap()
    # Transposed output out[p, n6, m] = result[m, 128*n6 + p]; host unscrambles.
    out = nc.dram_tensor("out", [128, NC6, M], _BF16, kind="ExternalOutput").ap()

    with tile.TileContext(nc) as tc:
        _body(tc, xt, w, bias, out)
    nc.compile()
    return nc


def _body(tc, xt, w, bias, out):
    import contextlib
    ctx = contextlib.ExitStack()
    with ctx:
        nc = tc.nc
        singles = ctx.enter_context(tc.tile_pool(name="singles", bufs=1))
        opool = ctx.enter_context(tc.tile_pool(name="o", bufs=4))
        pspool = ctx.enter_context(tc.tile_pool(name="ps", bufs=8, space="PSUM"))

        # Loads: one ring (sync), in exact consumption order -- both HWDGE
        # rings drain through the same 16 SDMA engines, so a second ring
        # only round-robins packets and delays what the PE needs first.
        # The first m-block's weights and activations are loaded as
        # interleaved per-kc chunk pairs so the first matmul can issue
        # ~2 us after loads begin and the PE then paces with the DMA
        # stream (the kc-outer loop below consumes chunk k while chunk
        # k+1 is in flight). Blocks 1-3 land well ahead of consumption.
        # Each dma_start costs ~0.6 us of fixed HWDGE issue time on top of
        # its data time, so the load chain uses graduated chunk sizes: a
        # small (w, x) pair for the earliest possible first matmul, then
        # 2-kc and 3-kc chunks to keep the issue count low while the PE
        # consumes the earlier chunks. bias rides after the mt0 chain (it
        # is only needed by the first activation, much later).
        bias_sb = singles.tile([128, NC6], _F32)
        w_sb = singles.tile([128, KC, D], _BF16)
        xt_sb = singles.tile([128, KC, M], _BF16)
        for k0, k1 in ((0, 1), (1, 3), (3, 6)):
            nc.sync.dma_start(out=w_sb[:, k0:k1, :], in_=w[:, k0:k1, :])
            nc.sync.dma_start(out=xt_sb[:, k0:k1, 0:MT],
                              in_=xt[:, k0:k1, 0:MT])
        nc.sync.dma_start(out=bias_sb[:], in_=bias[:])
        for mt in range(1, NMT):
            sl = np.s_[:, :, mt * MT:(mt + 1) * MT]
            nc.sync.dma_start(out=xt_sb[sl], in_=xt[sl])

        # A few matmuls on a zeroed tile bridge the PE from the entry
        # barrier to the first real matmul so the HAM clock gate's busy
        # window starts counting as early as possible.
        zeros = singles.tile([128, 512], _BF16)
        nc.vector.memset(zeros[:], 0.0)
        ps_warm = pspool.tile([128, 512], _F32, tag="ps")
        for _ in range(3):
            nc.tensor.matmul(ps_warm[:], zeros[:, :128], zeros[:],
                             start=True, stop=True)

        # First m-block: kc-outer with all 6 n-chunk PSUM groups live, so
        # each arriving (w, x) chunk pair immediately yields 6 matmuls.
        ps0 = [pspool.tile([128, 512], _F32, tag="ps", name=f"ps0_{i}")
               for i in range(NC6)]
        for kc in range(KC):
            for n6 in range(NC6):
                nc.tensor.matmul(ps0[n6][:, :MT],
                                 w_sb[:, kc, n6 * 128:(n6 + 1) * 128],
                                 xt_sb[:, kc, 0:MT],
                                 start=(kc == 0), stop=(kc == KC - 1))
        for n6 in range(NC6):
            o_sb = opool.tile([128, MT], _BF16, tag="o")
            nc.scalar.activation(o_sb[:], ps0[n6][:, :MT],
                                 mybir.ActivationFunctionType.Gelu,
                                 bias=bias_sb[:, n6:n6 + 1])
            nc.sync.dma_start(out=out[:, n6, 0:MT], in_=o_sb[:])

        # Remaining m-blocks: data is resident by now; n6-outer keeps the
        # activation/store pipeline finely paced.
        for mt in range(1, NMT):
            m0 = mt * MT
            for n6 in range(NC6):
                ps = pspool.tile([128, 512], _F32, tag="ps")
                for kc in range(KC):
                    nc.tensor.matmul(ps[:, :MT],
                                     w_sb[:, kc, n6 * 128:(n6 + 1) * 128],
                                     xt_sb[:, kc, m0:m0 + MT],
                                     start=(kc == 0), stop=(kc == KC - 1))
                o_sb = opool.tile([128, MT], _BF16, tag="o")
                nc.scalar.activation(o_sb[:], ps[:, :MT],
                                     mybir.ActivationFunctionType.Gelu,
                                     bias=bias_sb[:, n6:n6 + 1])
                nc.sync.dma_start(out=out[:, n6, m0:m0 + MT], in_=o_sb[:])


_NC_CACHE = {}


def _get_nc():
    if "nc" not in _NC_CACHE:
        _NC_CACHE["nc"] = _build()
    return _NC_CACHE["nc"]


def _prep_core_inputs(image, W_proj, b_proj):
    """Host-side layout prep: im2col + transpose + bf16, all permutations."""
    image = np.asarray(image, dtype=np.float32)
    assert image.shape == (B, H, W, C)
    img_bf = image.astype(ml_dtypes.bfloat16)
    # im2col (row-major patch order, matching the reference)
    pat = img_bf.reshape(B, NH, P, NW, P, C).transpose(0, 1, 3, 2, 4, 5)
    pat = np.ascontiguousarray(pat).reshape(B, NPATCH, K)

    w_bf = np.asarray(W_proj, dtype=np.float32).astype(ml_dtypes.bfloat16)
    w_dev = np.ascontiguousarray(w_bf.reshape(KC, 128, D).transpose(1, 0, 2))
    b_dev = np.ascontiguousarray(
        np.asarray(b_proj, dtype=np.float32).reshape(NC6, 128).T)

    in_maps = []
    for c in range(NCORES):
        x = pat[c * BPC:(c + 1) * BPC].reshape(M, K)
        # xt[p, kc, m] = x[m, 128*kc + p]
        xt = np.ascontiguousarray(x.reshape(M, KC, 128).transpose(2, 1, 0))
        in_maps.append({"xt": xt, "w": w_dev, "bias": b_dev})
    return in_maps


def _run(image, W_proj, b_proj, **spmd_kwargs):
    spmd_kwargs.pop("transpose_mode", None)
    in_maps = _prep_core_inputs(image, W_proj, b_proj)
    nc = _get_nc()
    res = run_bass_kernel_spmd(nc, in_maps, core_ids=list(range(NCORES)),
                               **spmd_kwargs)
    # device layout [p, n6, m] -> [m, 128*n6+p] -> [BPC, NPATCH, D] f32
    outs = [
        np.ascontiguousarray(res.results[c]["out"].transpose(2, 1, 0))
        .astype(np.float32).reshape(BPC, NPATCH, D)
        for c in range(NCORES)
    ]
    full = np.concatenate(outs, axis=0)
    return full, res


def kernel(image, W_proj, b_proj):
    full, _ = _run(image, W_proj, b_proj)
    return full


# revision 7
# speedup vs baseline: 1.0834x; 1.0834x over previous
"""Patch-embedding kernel for Trainium2, data-parallel over batch on 8 NeuronCores.

Reference computation (per image):
  patches = im2col(image, 16x16)            # [196, 768]
  out = gelu(patches @ W + b, exact)        # [196, 768]

Sharding: batch 64 -> 8 images per core; host concatenates per-core outputs.

Layout strategy: im2col is a pure permutation for stride-16 non-overlapping
patches, so the HOST performs im2col + transpose + bf16 cast and uploads
X^T in k-major chunk layout xt[p, kc, m] = X[m, 128*kc + p]. Every device
DMA is then a wide contiguous read (>=1.5 KB per partition line) -- this
removes the 192-byte-fragment im2col gather and the on-device xbar
transposes that dominated the previous version.

Matmul orientation: transposed output. For each 128-wide n-chunk,
  psum[n, m] = sum_kc W[k, n].T @ X^T[k, m]
with W chunks as the stationary operand (natural layout, uploaded
pre-chunked) and X^T as the bf16 moving operand. Benefits:
  - no bias matmuls: bias is per-PARTITION in this orientation, applied for
    free by ScalarE as gelu(psum + bias[p]) during the PSUM->SBUF pass
  - m-tiles of 392 (=1568/4) tile M exactly; lhsT is always full 128x128
  - output stored bf16 (halves store traffic); host transposes + upcasts.

Per-core loop: for each m-tile (392 rows), for each of 6 n-chunks,
accumulate 6 k-chunk matmuls into one PSUM bank, then ScalarE applies
exact GELU (+bias) writing bf16, and the result is DMA'd out. 24 PSUM
groups rotate through all 8 banks; X^T arrives in 12 half-chunks so the
first m-tile's matmuls start after ~1.8 MB of loads instead of 3.6 MB.
"""

import numpy as np
import ml_dtypes

import concourse.bass as bass
import concourse.tile as tile
import concourse.mybir as mybir
from concourse import bacc
from concourse.bass_utils import run_bass_kernel_spmd

P = 16
D = 768
B, H, W, C = 64, 224, 224, 3
NH = NW = 14
NPATCH = NH * NW            # 196
K = P * P * C               # 768
NCORES = 8
BPC = B // NCORES           # 8 images per core
M = BPC * NPATCH            # 1568 output rows per core
KC = K // 128               # 6 k-chunks
NC6 = D // 128              # 6 n-chunks
MT = 392                    # m-tile (1568 = 4*392)
NMT = M // MT               # 4 m-tiles
MH = M // 2                 # 784, half-M granularity for the X^T loads

_BF16 = mybir.dt.bfloat16
_F32 = mybir.dt.float32


def _build():
    nc = bacc.Bacc("TRN2", target_bir_lowering=False, debug=False,
                   num_devices=NCORES)
    # Host-prepared layouts (see _run): all reads/writes contiguous.
    xt = nc.dram_tensor("xt", [128, KC, M], _BF16, kind="ExternalInput").ap()
    w = nc.dram_tensor("w", [128, KC, D], _BF16, kind="ExternalInput").ap()
    bias = nc.dram_tensor("bias", [128, NC6], _F32, kind="ExternalInput").ap()
    # Transposed output out[p, n6, m] = result[m, 128*n6 + p]; host unscrambles.
    out = nc.dram_tensor("out", [128, NC6, M], _BF16, kind="ExternalOutput").ap()

    with tile.TileContext(nc) as tc:
        _body(tc, xt, w, bias, out)
    nc.compile()
    return nc


def _body(tc, xt, w, bias, out):
    import contextlib
    ctx = contextlib.ExitStack()
    with ctx:
        nc = tc.nc
        singles = ctx.enter_context(tc.tile_pool(name="singles", bufs=1))
        opool = ctx.enter_context(tc.tile_pool(name="o", bufs=4))
        pspool = ctx.enter_context(tc.tile_pool(name="ps", bufs=8, space="PSUM"))

        # Loads: one ring (sync), in exact consumption order -- both HWDGE
        # rings drain through the same 16 SDMA engines, so a second ring
        # only round-robins packets and delays what the PE needs first.
        # The first m-block's weights and activations are loaded as
        # interleaved per-kc chunk pairs so the first matmul can issue
        # ~2 us after loads begin and the PE then paces with the DMA
        # stream (the kc-outer loop below consumes chunk k while chunk
        # k+1 is in flight). Blocks 1-3 land well ahead of consumption.
        # Loads on the sync ring in exact consumption order: interleaved
        # per-kc (w, x) chunk pairs for the first m-block so the kc-outer
        # loop below consumes chunk k while chunk k+1 is in flight, then
        # whole blocks 1-3. bias goes on the scalar ring (only needed by
        # the first activation, much later).
        bias_sb = singles.tile([128, NC6], _F32)
        nc.scalar.dma_start(out=bias_sb[:], in_=bias[:])
        w_sb = singles.tile([128, KC, D], _BF16)
        xt_sb = singles.tile([128, KC, M], _BF16)
        for kc in range(KC):
            nc.sync.dma_start(out=w_sb[:, kc, :], in_=w[:, kc, :])
            nc.sync.dma_start(out=xt_sb[:, kc, 0:MT], in_=xt[:, kc, 0:MT])
        for mt in range(1, NMT):
            sl = np.s_[:, :, mt * MT:(mt + 1) * MT]
            nc.sync.dma_start(out=xt_sb[sl], in_=xt[sl])

        # Matmuls on a zeroed tile bridge the PE continuously from the
        # entry barrier to the first real matmul (~2.5 us of cold-rate
        # matmuls) so the HAM clock gate's ~3.4 us busy window elapses
        # during the load phase instead of during real work.
        zeros = singles.tile([128, 512], _BF16)
        nc.vector.memset(zeros[:], 0.0)
        ps_warm = pspool.tile([128, 512], _F32, tag="ps")
        for _ in range(7):
            nc.tensor.matmul(ps_warm[:], zeros[:, :128], zeros[:],
                             start=True, stop=True)

        # First m-block: kc-outer with all 6 n-chunk PSUM groups live, so
        # each arriving (w, x) chunk pair immediately yields 6 matmuls.
        ps0 = [pspool.tile([128, 512], _F32, tag="ps", name=f"ps0_{i}")
               for i in range(NC6)]
        for kc in range(KC):
            for n6 in range(NC6):
                nc.tensor.matmul(ps0[n6][:, :MT],
                                 w_sb[:, kc, n6 * 128:(n6 + 1) * 128],
                                 xt_sb[:, kc, 0:MT],
                                 start=(kc == 0), stop=(kc == KC - 1))
        for n6 in range(NC6):
            o_sb = opool.tile([128, MT], _BF16, tag="o")
            nc.scalar.activation(o_sb[:], ps0[n6][:, :MT],
                                 mybir.ActivationFunctionType.Gelu,
                                 bias=bias_sb[:, n6:n6 + 1])
            nc.sync.dma_start(out=out[:, n6, 0:MT], in_=o_sb[:])

        # Remaining m-blocks: data is resident by now; n6-outer keeps the
        # activation/store pipeline finely paced.
        for mt in range(1, NMT):
            m0 = mt * MT
            for n6 in range(NC6):
                ps = pspool.tile([128, 512], _F32, tag="ps")
                for kc in range(KC):
                    nc.tensor.matmul(ps[:, :MT],
                                     w_sb[:, kc, n6 * 128:(n6 + 1) * 128],
                                     xt_sb[:, kc, m0:m0 + MT],
                                     start=(kc == 0), stop=(kc == KC - 1))
                o_sb = opool.tile([128, MT], _BF16, tag="o")
                nc.scalar.activation(o_sb[:], ps[:, :MT],
                                     mybir.ActivationFunctionType.Gelu,
                                     bias=bias_sb[:, n6:n6 + 1])
                nc.sync.dma_start(out=out[:, n6, m0:m0 + MT], in_=o_sb[:])


_NC_CACHE = {}


def _get_nc():
    if "nc" not in _NC_CACHE:
        _NC_CACHE["nc"] = _build()
    return _NC_CACHE["nc"]


def _prep_core_inputs(image, W_proj, b_proj):
    """Host-side layout prep: im2col + transpose + bf16, all permutations."""
    image = np.asarray(image, dtype=np.float32)
    assert image.shape == (B, H, W, C)
    img_bf = image.astype(ml_dtypes.bfloat16)
    # im2col (row-major patch order, matching the reference)
    pat = img_bf.reshape(B, NH, P, NW, P, C).transpose(0, 1, 3, 2, 4, 5)
    pat = np.ascontiguousarray(pat).reshape(B, NPATCH, K)

    w_bf = np.asarray(W_proj, dtype=np.float32).astype(ml_dtypes.bfloat16)
    w_dev = np.ascontiguousarray(w_bf.reshape(KC, 128, D).transpose(1, 0, 2))
    b_dev = np.ascontiguousarray(
        np.asarray(b_proj, dtype=np.float32).reshape(NC6, 128).T)

    in_maps = []
    for c in range(NCORES):
        x = pat[c * BPC:(c + 1) * BPC].reshape(M, K)
        # xt[p, kc, m] = x[m, 128*kc + p]
        xt = np.ascontiguousarray(x.reshape(M, KC, 128).transpose(2, 1, 0))
        in_maps.append({"xt": xt, "w": w_dev, "bias": b_dev})
    return in_maps


def _run(image, W_proj, b_proj, **spmd_kwargs):
    spmd_kwargs.pop("transpose_mode", None)
    in_maps = _prep_core_inputs(image, W_proj, b_proj)
    nc = _get_nc()
    res = run_bass_kernel_spmd(nc, in_maps, core_ids=list(range(NCORES)),
                               **spmd_kwargs)
    # device layout [p, n6, m] -> [m, 128*n6+p] -> [BPC, NPATCH, D] f32
    outs = [
        np.ascontiguousarray(res.results[c]["out"].transpose(2, 1, 0))
        .astype(np.float32).reshape(BPC, NPATCH, D)
        for c in range(NCORES)
    ]
    full = np.concatenate(outs, axis=0)
    return full, res


def kernel(image, W_proj, b_proj):
    full, _ = _run(image, W_proj, b_proj)
    return full


# revision 9
# speedup vs baseline: 1.1021x; 1.0173x over previous
"""Patch-embedding kernel for Trainium2, data-parallel over batch on 8 NeuronCores.

Reference computation (per image):
  patches = im2col(image, 16x16)            # [196, 768]
  out = gelu(patches @ W + b, exact)        # [196, 768]

Sharding: batch 64 -> 8 images per core; host concatenates per-core outputs.

Layout strategy: im2col is a pure permutation for stride-16 non-overlapping
patches, so the HOST performs im2col + transpose + bf16 cast and uploads
X^T in k-major chunk layout xt[p, kc, m] = X[m, 128*kc + p]. Every device
DMA is then a wide contiguous read (>=1.5 KB per partition line) -- this
removes the 192-byte-fragment im2col gather and the on-device xbar
transposes that dominated the previous version.

Matmul orientation: transposed output. For each 128-wide n-chunk,
  psum[n, m] = sum_kc W[k, n].T @ X^T[k, m]
with W chunks as the stationary operand (natural layout, uploaded
pre-chunked) and X^T as the bf16 moving operand. Benefits:
  - no bias matmuls: bias is per-PARTITION in this orientation, applied for
    free by ScalarE as gelu(psum + bias[p]) during the PSUM->SBUF pass
  - m-tiles of 392 (=1568/4) tile M exactly; lhsT is always full 128x128
  - output stored bf16 (halves store traffic); host transposes + upcasts.

Per-core loop: for each m-tile (392 rows), for each of 6 n-chunks,
accumulate 6 k-chunk matmuls into one PSUM bank, then ScalarE applies
exact GELU (+bias) writing bf16, and the result is DMA'd out. 24 PSUM
groups rotate through all 8 banks; X^T arrives in 12 half-chunks so the
first m-tile's matmuls start after ~1.8 MB of loads instead of 3.6 MB.
"""

import numpy as np
import ml_dtypes

import concourse.bass as bass
import concourse.tile as tile
import concourse.mybir as mybir
from concourse import bacc
from concourse.bass_utils import run_bass_kernel_spmd

P = 16
D = 768
B, H, W, C = 64, 224, 224, 3
NH = NW = 14
NPATCH = NH * NW            # 196
K = P * P * C               # 768
NCORES = 8
BPC = B // NCORES           # 8 images per core
M = BPC * NPATCH            # 1568 output rows per core
KC = K // 128               # 6 k-chunks
NC6 = D // 128              # 6 n-chunks
MT = 392                    # interior m-tile width
# m-block split: small first block so the PE can start on ~1.5 MB of
# loads instead of ~1.8, and a small last block to shorten the tail.
MBLOCKS = (196, 392, 392, 392, 196)
MOFF = (0, 196, 588, 980, 1372)

_BF16 = mybir.dt.bfloat16
_F32 = mybir.dt.float32


def _build():
    nc = bacc.Bacc("TRN2", target_bir_lowering=False, debug=False,
                   num_devices=NCORES)
    # Host-prepared layouts (see _run): all reads/writes contiguous.
    xt = nc.dram_tensor("xt", [128, KC, M], _BF16, kind="ExternalInput").ap()
    w = nc.dram_tensor("w", [128, KC, D], _BF16, kind="ExternalInput").ap()
    bias = nc.dram_tensor("bias", [128, NC6], _F32, kind="ExternalInput").ap()
    # Transposed output out[p, n6, m] = result[m, 128*n6 + p]; host unscrambles.
    out = nc.dram_tensor("out", [128, NC6, M], _BF16, kind="ExternalOutput").ap()

    with tile.TileContext(nc) as tc:
        _body(tc, xt, w, bias, out)
    nc.compile()
    return nc


def _body(tc, xt, w, bias, out):
    import contextlib
    ctx = contextlib.ExitStack()
    with ctx:
        nc = tc.nc
        singles = ctx.enter_context(tc.tile_pool(name="singles", bufs=1))
        opool = ctx.enter_context(tc.tile_pool(name="o", bufs=4))
        pspool = ctx.enter_context(tc.tile_pool(name="ps", bufs=8, space="PSUM"))

        # Loads: one ring (sync), in exact consumption order -- both HWDGE
        # rings drain through the same 16 SDMA engines, so a second ring
        # only round-robins packets and delays what the PE needs first.
        # The first m-block's weights and activations are loaded as
        # interleaved per-kc chunk pairs so the first matmul can issue
        # ~2 us after loads begin and the PE then paces with the DMA
        # stream (the kc-outer loop below consumes chunk k while chunk
        # k+1 is in flight). Blocks 1-3 land well ahead of consumption.
        # Loads on the sync ring in exact consumption order. The first
        # m-block's dependencies (W + 0.3 MB of X^T) are interleaved so
        # the kc-outer loop below consumes chunk k while chunk k+1 is in
        # flight; later blocks stream while the PE chews. bias goes on
        # the scalar ring (only needed by the first activation).
        bias_sb = singles.tile([128, NC6], _F32)
        nc.scalar.dma_start(out=bias_sb[:], in_=bias[:])
        w_sb = singles.tile([128, KC, D], _BF16)
        xt_sb = singles.tile([128, KC, M], _BF16)
        B0 = MBLOCKS[0]
        nc.sync.dma_start(out=w_sb[:, 0:2, :], in_=w[:, 0:2, :])
        nc.sync.dma_start(out=xt_sb[:, 0:3, 0:B0], in_=xt[:, 0:3, 0:B0])
        nc.sync.dma_start(out=w_sb[:, 2:4, :], in_=w[:, 2:4, :])
        nc.sync.dma_start(out=xt_sb[:, 3:6, 0:B0], in_=xt[:, 3:6, 0:B0])
        nc.sync.dma_start(out=w_sb[:, 4:6, :], in_=w[:, 4:6, :])
        for mb in range(1, len(MBLOCKS)):
            sl = np.s_[:, :, MOFF[mb]:MOFF[mb] + MBLOCKS[mb]]
            nc.sync.dma_start(out=xt_sb[sl], in_=xt[sl])

        # Matmuls on a zeroed tile bridge the PE continuously from the
        # entry barrier to the first real matmul so the HAM clock gate's
        # ~3.4 us busy window elapses during the load phase instead of
        # during real work.
        zeros = singles.tile([128, 512], _BF16)
        nc.vector.memset(zeros[:], 0.0)
        ps_warm = pspool.tile([128, 512], _F32, tag="ps")
        for _ in range(5):
            nc.tensor.matmul(ps_warm[:], zeros[:, :128], zeros[:],
                             start=True, stop=True)

        # First m-block: kc-outer with all 6 n-chunk PSUM groups live, so
        # each arriving (w, x) chunk pair immediately yields 6 matmuls.
        ps0 = [pspool.tile([128, 512], _F32, tag="ps", name=f"ps0_{i}")
               for i in range(NC6)]
        for kc in range(KC):
            for n6 in range(NC6):
                nc.tensor.matmul(ps0[n6][:, :B0],
                                 w_sb[:, kc, n6 * 128:(n6 + 1) * 128],
                                 xt_sb[:, kc, 0:B0],
                                 start=(kc == 0), stop=(kc == KC - 1))
        for n6 in range(NC6):
            o_sb = opool.tile([128, B0], _BF16, tag="o0")
            nc.scalar.activation(o_sb[:], ps0[n6][:, :B0],
                                 mybir.ActivationFunctionType.Gelu,
                                 bias=bias_sb[:, n6:n6 + 1])
            nc.sync.dma_start(out=out[:, n6, 0:B0], in_=o_sb[:])

        # Remaining m-blocks: data is resident (or lands just ahead);
        # n6-outer keeps the activation/store pipeline finely paced.
        for mb in range(1, len(MBLOCKS)):
            m0, mw = MOFF[mb], MBLOCKS[mb]
            for n6 in range(NC6):
                ps = pspool.tile([128, 512], _F32, tag="ps")
                for kc in range(KC):
                    nc.tensor.matmul(ps[:, :mw],
                                     w_sb[:, kc, n6 * 128:(n6 + 1) * 128],
                                     xt_sb[:, kc, m0:m0 + mw],
                                     start=(kc == 0), stop=(kc == KC - 1))
                o_sb = opool.tile([128, mw], _BF16, tag=f"o{mw}")
                nc.scalar.activation(o_sb[:], ps[:, :mw],
                                     mybir.ActivationFunctionType.Gelu,
                                     bias=bias_sb[:, n6:n6 + 1])
                nc.sync.dma_start(out=out[:, n6, m0:m0 + mw], in_=o_sb[:])


_NC_CACHE = {}


def _get_nc():
    if "nc" not in _NC_CACHE:
        _NC_CACHE["nc"] = _build()
    return _NC_CACHE["nc"]


def _prep_core_inputs(image, W_proj, b_proj):
    """Host-side layout prep: im2col + transpose + bf16, all permutations."""
    image = np.asarray(image, dtype=np.float32)
    assert image.shape == (B, H, W, C)
    img_bf = image.astype(ml_dtypes.bfloat16)
    # im2col (row-major patch order, matching the reference)
    pat = img_bf.reshape(B, NH, P, NW, P, C).transpose(0, 1, 3, 2, 4, 5)
    pat = np.ascontiguousarray(pat).reshape(B, NPATCH, K)

    w_bf = np.asarray(W_proj, dtype=np.float32).astype(ml_dtypes.bfloat16)
    w_dev = np.ascontiguousarray(w_bf.reshape(KC, 128, D).transpose(1, 0, 2))
    b_dev = np.ascontiguousarray(
        np.asarray(b_proj, dtype=np.float32).reshape(NC6, 128).T)

    in_maps = []
    for c in range(NCORES):
        x = pat[c * BPC:(c + 1) * BPC].reshape(M, K)
        # xt[p, kc, m] = x[m, 128*kc + p]
        xt = np.ascontiguousarray(x.reshape(M, KC, 128).transpose(2, 1, 0))
        in_maps.append({"xt": xt, "w": w_dev, "bias": b_dev})
    return in_maps


def _run(image, W_proj, b_proj, **spmd_kwargs):
    spmd_kwargs.pop("transpose_mode", None)
    in_maps = _prep_core_inputs(image, W_proj, b_proj)
    nc = _get_nc()
    res = run_bass_kernel_spmd(nc, in_maps, core_ids=list(range(NCORES)),
                               **spmd_kwargs)
    # device layout [p, n6, m] -> [m, 128*n6+p] -> [BPC, NPATCH, D] f32
    outs = [
        np.ascontiguousarray(res.results[c]["out"].transpose(2, 1, 0))
        .astype(np.float32).reshape(BPC, NPATCH, D)
        for c in range(NCORES)
    ]
    full = np.concatenate(outs, axis=0)
    return full, res


def kernel(image, W_proj, b_proj):
    full, _ = _run(image, W_proj, b_proj)
    return full
